# revision 1
# baseline (speedup 1.0000x reference)
"""KMeans assignment kernel (retrieval_knn) for 8 Trainium2 NeuronCores.

Computes argmin_k ||x_n - c_k||^2 for x [262144, 64] f32 against
centers [1024, 64] f32, returning int32 cluster ids [262144].

argmin ||x-c||^2 == argmax s, s = 2x.c - ||c||^2, computed on the PE via
bf16 hi/lo split matmuls (near-fp32). ScalarE copies PSUM->SBUF; DVE does
ONE segmented-max pass (64 group maxima/tile); batched equality+iota ops
pick the winning group; scores are spilled to raw DRAM tensors and an
indirect DMA gathers just the winning 16-el group per point; a 16-wide
max_index gives the position -> id = group*16 + pos.  (Gather source must
be a raw nc.dram_tensor — DRAM pool tiles break indirect DMA.)
"""

import numpy as np
import ml_dtypes

N_POINTS = 262144
N_FEATURES = 64
N_CLUSTERS = 1024
N_CORES = 8
PTS_PER_CORE = N_POINTS // N_CORES      # 32768
TILE_P = 128                            # points per tile (partition dim)
N_TILES = PTS_PER_CORE // TILE_P        # 256
KH = 512                                # centers per matmul chunk

_CACHE = {}


def _build_bass():
    import concourse.bass as bass
    import concourse.bacc as bacc
    import concourse.mybir as mybir
    import concourse.tile as tile
    from contextlib import ExitStack

    bf16 = mybir.dt.bfloat16
    f32 = mybir.dt.float32
    u32 = mybir.dt.uint32

    nc = bacc.Bacc(None, target_bir_lowering=False)

    xpack = nc.declare_dram_parameter("xpack", [128, PTS_PER_CORE], bf16, isOutput=False)
    cc = nc.declare_dram_parameter("cc", [128, N_CLUSTERS], bf16, isOutput=False)
    cloa = nc.declare_dram_parameter("cloa", [67, N_CLUSTERS], bf16, isOutput=False)
    tc64 = nc.declare_dram_parameter("tc64", [128, 8], f32, isOutput=False)
    out = nc.declare_dram_parameter("out", [128, N_TILES], u32, isOutput=True)

    BT = 8            # tiles per stage-2 batch
    G = 64            # groups per tile
    GS = 16           # group size (elements gathered per point)

    # raw DRAM spill buffers (manual double buffer, alternating per batch)
    spills = [
        nc.dram_tensor(f"sspill{j}", [128 * BT * G, GS], f32) for j in range(2)
    ]

    with tile.TileContext(nc) as tc, ExitStack() as ctx:
        const_pool = ctx.enter_context(tc.tile_pool(name="const", bufs=1))
        xin_pool = ctx.enter_context(tc.tile_pool(name="xin", bufs=3))
        xa_pool = ctx.enter_context(tc.tile_pool(name="xa", bufs=3))
        psum_pool = ctx.enter_context(
            tc.tile_pool(name="psum", bufs=4, space=bass.MemorySpace.PSUM)
        )
        s_pool = ctx.enter_context(tc.tile_pool(name="s", bufs=4))
        batch_pool = ctx.enter_context(tc.tile_pool(name="batch", bufs=3))
        small_pool = ctx.enter_context(tc.tile_pool(name="small", bufs=6))
        gv_pool = ctx.enter_context(tc.tile_pool(name="gv", bufs=10))
        out_pool = ctx.enter_context(tc.tile_pool(name="out", bufs=1))

        cc_t = const_pool.tile([128, N_CLUSTERS], bf16)
        nc.sync.dma_start(cc_t[:], cc[:])
        cloa_t = const_pool.tile([67, N_CLUSTERS], bf16)
        nc.sync.dma_start(cloa_t[:], cloa[:])
        tc64_t = const_pool.tile([128, 8], f32)
        nc.sync.dma_start(tc64_t[:], tc64[:])

        outbuf = out_pool.tile([128, N_TILES], u32)

        XB = 4  # tiles per x load / score-spill batch
        for tb in range(N_TILES // BT):
            maB = batch_pool.tile([128, BT, G], f32)
            spillb = spills[tb % 2]
            spillb_w = spillb[:].rearrange(
                "(p i g) e -> p i (g e)", p=128, i=BT
            )
            for i in range(BT):
                t = tb * BT + i
                if t % XB == 0:
                    xp = xin_pool.tile([128, XB, TILE_P], bf16)
                    csl = slice(t * TILE_P, (t + XB) * TILE_P)
                    nc.sync.dma_start(
                        xp[:], xpack[:, csl].rearrange("p (b q) -> p b q", b=XB)
                    )
                    # second copy of the xhi rows with 3 all-ones aug rows
                    # (stationary for the xhi.clo - cn matmul)
                    xa = xa_pool.tile([67, XB, TILE_P], bf16)
                    nc.sync.dma_start(
                        xa[0:64],
                        xpack[0:64, csl].rearrange("p (b q) -> p b q", b=XB),
                    )
                    nc.gpsimd.memset(xa[64:67], 1.0)
                xi = t % XB
                ps = psum_pool.tile([128, N_CLUSTERS], f32)
                for kh in range(N_CLUSTERS // KH):
                    ksl = slice(kh * KH, (kh + 1) * KH)
                    nc.tensor.matmul(
                        ps[:, ksl], xp[:, xi, :], cc_t[:, ksl],
                        start=True, stop=False,
                    )
                    nc.tensor.matmul(
                        ps[:, ksl], xa[:, xi, :], cloa_t[:, ksl],
                        start=False, stop=True,
                    )
                if i % XB == 0:
                    s4 = s_pool.tile([128, XB, N_CLUSTERS], f32)
                si = i % XB
                for kh in range(N_CLUSTERS // KH):
                    ksl = slice(kh * KH, (kh + 1) * KH)
                    nc.scalar.copy(s4[:, si, ksl], ps[:, ksl])
                # stage 1: segmented max over 64 groups of 16
                nc.vector.tensor_reduce(
                    maB[:, i, :],
                    s4[:, si, :].rearrange("p (g e) -> p g e", g=G),
                    axis=mybir.AxisListType.X,
                    op=mybir.AluOpType.max,
                )
                if i % XB == XB - 1:
                    # spill 4 tiles of scores in one DMA, alternating the
                    # issuing queue (transfer time lands on the issuer)
                    eng = nc.gpsimd if (t // XB) % 2 == 0 else nc.sync
                    eng.dma_start(spillb_w[:, i - (XB - 1) : i + 1, :], s4[:])

            # stage 2 (batched): per-tile max value and winning group index
            m8b = small_pool.tile([128, BT], f32)
            nc.vector.tensor_reduce(
                m8b[:], maB[:], axis=mybir.AxisListType.X, op=mybir.AluOpType.max
            )
            gw = small_pool.tile([128, BT, 8], u32)
            for i in range(BT):
                nc.vector.max_index(
                    gw[:, i, :],
                    m8b[:, i : i + 1].to_broadcast([128, 8]),
                    maB[:, i, :],
                )
            g8 = small_pool.tile([128, BT], f32)
            nc.vector.tensor_copy(g8[:], gw[:, :, 0])
            # gather row index = p*(BT*G) + i*G + g  (tc64 holds the p,i part)
            offf = small_pool.tile([128, BT], f32)
            nc.vector.tensor_tensor(
                offf[:], g8[:], tc64_t[:], op=mybir.AluOpType.add
            )
            offu = small_pool.tile([128, BT], u32)
            nc.vector.tensor_copy(offu[:], offf[:])
            # stage 3: gather each tile's winning 16-el group, then find the
            # max's position within it
            jw = small_pool.tile([128, BT, 8], u32)
            for i in range(BT):
                gv = gv_pool.tile([128, GS], f32)
                nc.gpsimd.indirect_dma_start(
                    out=gv[:],
                    out_offset=None,
                    in_=spillb[:],
                    in_offset=bass.IndirectOffsetOnAxis(
                        ap=offu[:, i : i + 1], axis=0
                    ),
                )
                nc.vector.max_index(
                    jw[:, i, :],
                    m8b[:, i : i + 1].to_broadcast([128, 8]),
                    gv[:],
                )
            jf = small_pool.tile([128, BT], f32)
            nc.vector.tensor_copy(jf[:], jw[:, :, 0])
            g16 = small_pool.tile([128, BT], f32)
            nc.vector.tensor_scalar_mul(g16[:], g8[:], float(GS))
            idxf = small_pool.tile([128, BT], f32)
            nc.vector.tensor_tensor(
                idxf[:], g16[:], jf[:], op=mybir.AluOpType.add
            )
            nc.vector.tensor_copy(outbuf[:, tb * BT : (tb + 1) * BT], idxf[:])

        nc.sync.dma_start(out[:], outbuf[:])

    nc.compile()
    return nc


def _prep(x: np.ndarray, centers: np.ndarray):
    bf16 = ml_dtypes.bfloat16
    xt = np.ascontiguousarray(x.T)                      # [64, N] f32
    xhi = xt.astype(bf16)
    xlo = (xt - xhi.astype(np.float32)).astype(bf16)
    xpack = np.concatenate([xhi, xlo], axis=0)          # [128, N] bf16

    c2t = np.ascontiguousarray((2.0 * centers).T)       # [64, K] f32
    chi = c2t.astype(bf16)
    clo = (c2t - chi.astype(np.float32)).astype(bf16)   # [64, K] bf16
    cc = np.concatenate([chi, chi], axis=0)             # [128, K] bf16

    # -||c||^2 as a 3-term bf16 cascade, matched with all-ones stationary rows
    cn = np.sum(centers.astype(np.float32) ** 2, axis=1, dtype=np.float32)
    n1 = (-cn).astype(bf16)
    r1 = -cn - n1.astype(np.float32)
    n2 = r1.astype(bf16)
    n3 = (r1 - n2.astype(np.float32)).astype(bf16)
    cloa = np.concatenate(
        [clo, n1[None, :], n2[None, :], n3[None, :]], axis=0
    )                                                   # [67, K] bf16

    p = np.arange(128, dtype=np.float32)[:, None]
    i = np.arange(8, dtype=np.float32)[None, :]
    tc64 = np.ascontiguousarray(p * (8 * 64.0) + i * 64.0)
    return xpack, cc, cloa, tc64


def kernel(x: np.ndarray, centers: np.ndarray) -> np.ndarray:
    import sys
    if "/opt/trn_rl_repo" not in sys.path:
        sys.path.insert(0, "/opt/trn_rl_repo")
    from concourse.bass_utils import run_bass_kernel_spmd

    x = np.asarray(x, dtype=np.float32)
    centers = np.asarray(centers, dtype=np.float32)

    xpack, cc, cloa, tc64 = _prep(x, centers)

    if "nc" not in _CACHE:
        _CACHE["nc"] = _build_bass()
    nc = _CACHE["nc"]

    in_maps = []
    for c in range(N_CORES):
        sl = slice(c * PTS_PER_CORE, (c + 1) * PTS_PER_CORE)
        in_maps.append(
            {
                "xpack": np.ascontiguousarray(xpack[:, sl]),
                "cc": cc,
                "cloa": cloa,
                "tc64": tc64,
            }
        )

    res = run_bass_kernel_spmd(nc, in_maps, list(range(N_CORES)))

    outs = []
    for c in range(N_CORES):
        o = res.results[c]["out"]                       # [128, N_TILES] uint32
        outs.append(np.asarray(o).astype(np.int64).T.reshape(-1))  # point t*128+p
    ids = np.concatenate(outs)
    return ids.astype(np.int32)


if __name__ == "__main__":
    rng = np.random.default_rng(0)
    x = rng.normal(size=(N_POINTS, N_FEATURES)).astype(np.float32)
    c = rng.normal(size=(N_CLUSTERS, N_FEATURES)).astype(np.float32)
    ids = kernel(x=x, centers=c)
    d = (
        np.sum(x * x, 1)[:, None]
        - 2.0 * (x @ c.T)
        + np.sum(c * c, 1)[None, :]
    )
    ref = np.argmin(np.abs(d), axis=1)
    print("mismatch:", np.mean(ids != ref))

